# revision 16
# baseline (speedup 1.0000x reference)
"""AGDN (2-layer GAT-style message passing) distributed Bass kernel for 8 TRN2 cores.

v2: dst-sharded edge processing, bf16 gather tables (256B rows), batched
dma_gather calls (B tiles per ucode launch), attention-r broadcast via PE
transpose + one-hot matmul (no per-slot gather), residual+bias prefilled in
DRAM with a dma_scatter_add epilogue (no per-tile residual gather, no
indirect scatter).

Table row (128 bf16 = 256B): [x_lin(64, head-interleaved) | ones(H) | al(H) |
ar(H) | pad].  Edges are packed host-side into R=4 slots sharing (dst,
src-quarter); tiles cover <=128 dst nodes x 4 quarters x 128 slots.
"""

import numpy as np

import concourse.bass as bass
import concourse.bacc as bacc
import concourse.mybir as mybir
import concourse.tile as tile
from concourse import bass_utils

F32 = mybir.dt.float32
BF16 = mybir.dt.bfloat16
I16 = mybir.dt.int16
AF = mybir.ActivationFunctionType
OP = mybir.AluOpType
AX = mybir.AxisListType

# problem constants
IN, HID, HEADS, OUT = 128, 16, 4, 64
SLOPE = 0.2
NC = 8
NQ = 4                  # src quarters (2 cores each)
R = 4                   # edges per slot (same dst, same src-quarter)
GROUPS = 4
BLOCKS = GROUPS * R
ROW = 128               # table row bf16 elems (256B)
B = 4                   # tiles per super-tile (gather batching)

_CACHE = {}


def _set_dims(n, e):
    global N, E, NLOC, NPAD, HBLK, NBLK, QROWS
    N, E = n, e
    NLOC = N // NC
    NPAD = NLOC + 4
    HBLK = ((NPAD + 127) // 128) * 128
    NBLK = HBLK // 128
    QROWS = 2 * NPAD
    assert QROWS < 32768


_set_dims(100000, 1600000)

# aux16 column layout (per super-tile)
CA_X = 4 * B * 32                 # X gather idxs, quarter-major
CA_AR = CA_X + B * 8              # ar-tile gather idxs
CA_OUT = CA_AR + B * 8            # scatter-add out idxs
CA16 = CA_OUT


def _wrap16(idx):
    """[n] -> int16 [128, n//16] replicated-8 layout for dma_gather."""
    n = idx.shape[0]
    blk = idx.reshape(n // 16, 16).T.astype(np.int16)
    return np.tile(blk, (8, 1))


# --------------------------------------------------------------------------
# host-side graph packing
# --------------------------------------------------------------------------

def _pack_core(src_g, dst_l):
    quarter = src_g // (2 * NLOC)
    key = dst_l * NQ + quarter
    korder = np.argsort(key, kind="stable")
    src_g = src_g[korder]
    key = key[korder]
    cnt = np.bincount(key, minlength=NLOC * NQ)
    estart = np.concatenate([[0], np.cumsum(cnt)])[:-1]
    nslot_nq = ((cnt + R - 1) // R).reshape(NLOC, NQ)

    cum_q = np.cumsum(nslot_nq, axis=0)
    tiles = []
    n0 = 0
    base = np.zeros(NQ, np.int64)
    while n0 < NLOC:
        n_hi = min(n0 + 128, NLOC)
        ok = ((cum_q[n0:n_hi] - base[None, :]) <= 128).all(axis=1)
        k = int(np.argmin(ok)) if not ok.all() else n_hi - n0
        n1 = n0 + max(k, 1)
        if (nslot_nq[n0] > 128).any():
            raise ValueError("node with too many edges in one quarter")
        tiles.append((n0, n1))
        base = cum_q[n1 - 1].copy()
        n0 = n1
    T = len(tiles)

    zq = NLOC   # quarter-local / local zero row
    srcidx = np.full((T, GROUPS, R, 128), zq, np.int64)
    ld = np.full((T, 128, GROUPS), 255.0, np.float32)
    outl = np.zeros((T, 128), np.int64)
    for t, (nlo, nhi) in enumerate(tiles):
        for g in range(NQ):
            p = 0
            for d in range(nlo, nhi):
                ns = nslot_nq[d, g]
                if ns == 0:
                    continue
                e0 = estart[d * NQ + g]
                ne = cnt[d * NQ + g]
                for s in range(ns):
                    tk = min(R, ne - R * s)
                    rows = src_g[e0 + R * s : e0 + R * s + tk]
                    srcidx[t, g, :tk, p] = (rows // NLOC % 2) * NPAD + rows % NLOC
                    ld[t, p, g] = d - nlo
                    p += 1
            assert p <= 128
        out_l = np.arange(128) + nlo
        pad = out_l >= nhi
        out_l[pad] = NLOC + (np.arange(128)[pad] % 4)
        outl[t] = out_l
    return srcidx, ld, outl, T


def _prepare(edge_index):
    src = edge_index[0].astype(np.int64)
    dst = edge_index[1].astype(np.int64)
    core = dst // NLOC
    packs = []
    for c in range(NC):
        m = core == c
        packs.append(_pack_core(src[m], dst[m] - c * NLOC))
    Tmax = max(p[3] for p in packs)
    TS = (Tmax + B - 1) // B
    Tp = TS * B
    a16 = np.zeros((NC, TS, 128, CA16), np.int16)
    af = np.zeros((NC, TS, 128, B * GROUPS), np.float32)
    for c in range(NC):
        srcidx, ld, outl, T = packs[c]
        # pad to Tp tiles
        si = np.full((Tp, GROUPS, R, 128), NLOC, np.int64)
        lf = np.full((Tp, 128, GROUPS), 255.0, np.float32)
        ol = np.full((Tp, 128), NLOC, np.int64)
        si[:T] = srcidx
        lf[:T] = ld
        ol[:T] = outl
        for s in range(TS):
            for g in range(GROUPS):
                # idx order: flat j = (i*R + r)*128 + p  -> out[p, i*R+r, :]
                flat = si[s * B : (s + 1) * B, g].reshape(-1)
                a16[c, s, :, g * B * 32 : (g + 1) * B * 32] = _wrap16(flat)
            a16[c, s, :, CA_X:CA_AR] = _wrap16(ol[s * B : (s + 1) * B].reshape(-1))
            a16[c, s, :, CA_AR:CA_OUT] = _wrap16(ol[s * B : (s + 1) * B].reshape(-1))
            af[c, s] = lf[s * B : (s + 1) * B].transpose(1, 0, 2).reshape(128, B * GROUPS)
    return a16, af, TS


# --------------------------------------------------------------------------
# device kernel
# --------------------------------------------------------------------------

def _build(TS):
    nc = bacc.Bacc("TRN2", target_bir_lowering=False, num_devices=NC,
                   num_swdge_queues=4)

    xT = nc.declare_dram_parameter("xT", [NBLK * 128, 128], BF16, isOutput=False)
    aux16_d = nc.declare_dram_parameter("aux16", [TS, 128, CA16], I16, isOutput=False)
    auxf_d = nc.declare_dram_parameter("auxf", [TS, 128, B * GROUPS], F32, isOutput=False)
    w1 = nc.declare_dram_parameter("w1", [IN, OUT], BF16, isOutput=False)
    rw1 = nc.declare_dram_parameter("rw1", [IN, OUT], BF16, isOutput=False)
    atl1 = nc.declare_dram_parameter("atl1", [128, OUT], F32, isOutput=False)
    atr1 = nc.declare_dram_parameter("atr1", [128, OUT], F32, isOutput=False)
    b1 = nc.declare_dram_parameter("b1", [128, OUT], F32, isOutput=False)
    w2 = nc.declare_dram_parameter("w2", [OUT, OUT], BF16, isOutput=False)
    atl2 = nc.declare_dram_parameter("atl2", [128, OUT], F32, isOutput=False)
    atr2 = nc.declare_dram_parameter("atr2", [128, OUT], F32, isOutput=False)
    b2 = nc.declare_dram_parameter("b2", [128, OUT], F32, isOutput=False)
    iota_d = nc.declare_dram_parameter("iota", [128, 128], F32, isOutput=False)
    ident_d = nc.declare_dram_parameter("ident", [128, 128], BF16, isOutput=False)
    out_d = nc.declare_dram_parameter("out", [NPAD, OUT], F32, isOutput=True)

    CHB = [(0, 24), (24, 48), (48, 72), (72, NBLK)]   # AllGather chunks (blocks)

    with tile.TileContext(nc) as tc:
        with (
            tc.tile_pool(name="dram", bufs=1, space="DRAM") as dram,
            tc.tile_pool(name="const", bufs=1) as cpool,
            tc.tile_pool(name="sba", bufs=3) as sba,
            tc.tile_pool(name="psum", bufs=2, space="PSUM") as pp,
            tc.tile_pool(name="sbb", bufs=3) as sbb,
        ):
            shard1 = dram.tile([NPAD, ROW], BF16)
            t1 = dram.tile([NC * NPAD, ROW], BF16)
            h_l = dram.tile([HBLK, OUT], F32)
            shard2 = dram.tile([NPAD, ROW], BF16)
            t2 = dram.tile([NC * NPAD, ROW], BF16)

            def const_from(handle, shape, tag, dt=F32):
                t_ = cpool.tile(shape, dt, tag=tag, name=tag)
                nc.sync.dma_start(t_[:], handle[:])
                return t_

            w1_sb = const_from(w1, [IN, OUT], "c_w1", BF16)
            rw1_sb = const_from(rw1, [IN, OUT], "c_rw1", BF16)
            w2_sb = const_from(w2, [OUT, OUT], "c_w2", BF16)
            atl1_sb = const_from(atl1, [128, OUT], "c_atl1")
            atr1_sb = const_from(atr1, [128, OUT], "c_atr1")
            b1_sb = const_from(b1, [128, OUT], "c_b1")
            atl2_sb = const_from(atl2, [128, OUT], "c_atl2")
            atr2_sb = const_from(atr2, [128, OUT], "c_atr2")
            b2_sb = const_from(b2, [128, OUT], "c_b2")
            iota_sb = const_from(iota_d, [128, 128], "c_iota")
            ident_sb = const_from(ident_d, [128, 128], "c_ident", BF16)
            zrow_sb = cpool.tile([4, ROW], BF16)
            nc.vector.memset(zrow_sb[:], 0.0)
            ztail_sb = cpool.tile([HBLK - NLOC, OUT], F32)
            nc.vector.memset(ztail_sb[:], 0.0)

            def ag_chunk(shard, tdst, blo, bhi, last, tag):
                a = blo * 128
                b_ = NPAD if last else bhi * 128
                rows = b_ - a
                tmp = dram.tile([NC * rows, ROW], BF16, tag=f"agt{tag}",
                                name=f"agt{tag}")
                nc.gpsimd.collective_compute(
                    "AllGather", OP.bypass,
                    replica_groups=[list(range(NC))],
                    ins=[shard[a:b_, :].opt()], outs=[tmp.opt()],
                )
                for c in range(NC):
                    nc.sync.dma_start(
                        tdst[c * NPAD + a : c * NPAD + b_, :],
                        tmp[c * rows : (c + 1) * rows, :])

            def store2(dst, buf, k0, width):
                # buf [128, 2, width] -> dst rows [k0*128 : k0*128+256) (clip NLOC)
                lo = k0 * 128
                if lo + 256 <= NLOC:
                    nc.sync.dma_start(
                        dst[lo : lo + 256, :].rearrange("(j p) w -> p j w", j=2),
                        buf[:],
                    )
                else:
                    r0 = min(128, NLOC - lo)
                    nc.sync.dma_start(dst[lo : lo + r0, :], buf[:r0, 0, :])
                    r1 = min(128, NLOC - lo - 128)
                    if r1 > 0:
                        nc.sync.dma_start(
                            dst[lo + 128 : lo + 128 + r1, :], buf[:r1, 1, :])

            # ---- phase A: x @ W1, attention dots, residual prefill ----
            for j in range(NBLK // 2):
                k0 = 2 * j
                xk = sba.tile([128, 2, 128], BF16, tag="xk")
                nc.sync.dma_start(
                    xk[:],
                    xT[k0 * 128 : (k0 + 2) * 128, :].rearrange(
                        "(j p) c -> p j c", j=2),
                )
                pa = pp.tile([128, 4, OUT], F32, tag="pa")
                for jj in range(2):
                    nc.tensor.matmul(pa[:, jj], lhsT=xk[:, jj], rhs=w1_sb[:],
                                     start=True, stop=True)
                    nc.tensor.matmul(pa[:, 2 + jj], lhsT=xk[:, jj], rhs=rw1_sb[:],
                                     start=True, stop=True)
                sh = sba.tile([128, 2, ROW], BF16, tag="sh")
                nc.scalar.activation(sh[:, :, 0:OUT], pa[:, 0:2], AF.Copy)
                nc.vector.memset(sh[:, :, OUT : OUT + HEADS], 1.0)
                tal = sba.tile([128, 2, OUT], F32, tag="tal")
                nc.vector.tensor_tensor(
                    tal[:], pa[:, 0:2],
                    atl1_sb[:, None, :].broadcast_to([128, 2, OUT]), OP.mult)
                alr = sba.tile([128, 2, 2 * HEADS], F32, tag="alr")
                nc.vector.tensor_reduce(
                    alr[:, :, 0:HEADS],
                    tal[:].rearrange("p j (c h) -> p j h c", h=HEADS),
                    AX.X, OP.add)
                tar = sba.tile([128, 2, OUT], F32, tag="tar")
                nc.vector.tensor_tensor(
                    tar[:], pa[:, 0:2],
                    atr1_sb[:, None, :].broadcast_to([128, 2, OUT]), OP.mult)
                nc.vector.tensor_reduce(
                    alr[:, :, HEADS : 2 * HEADS],
                    tar[:].rearrange("p j (c h) -> p j h c", h=HEADS),
                    AX.X, OP.add)
                nc.scalar.activation(
                    sh[:, :, OUT + HEADS : OUT + 3 * HEADS], alr[:], AF.Copy)
                xr = sba.tile([128, 2, OUT], F32, tag="xr")
                nc.vector.tensor_tensor(
                    xr[:], pa[:, 2:4],
                    b1_sb[:, None, :].broadcast_to([128, 2, OUT]), OP.add)
                store2(shard1, sh, k0, ROW)
                lo = k0 * 128
                nc.sync.dma_start(
                    h_l[lo : lo + 256, :].rearrange("(j p) w -> p j w", j=2),
                    xr[:])
                for ci, (blo, bhi) in enumerate(CHB[:-1]):
                    if k0 + 2 == bhi:
                        ag_chunk(shard1, t1, blo, bhi, False, f"1_{ci}")
            nc.sync.dma_start(shard1[NLOC:NPAD, :], zrow_sb[:, 0:ROW])
            nc.sync.dma_start(h_l[NLOC:HBLK, :], ztail_sb[:])
            ag_chunk(shard1, t1, CHB[-1][0], CHB[-1][1], True, '1_3')

            _edge_layer(nc, sbb, pp, aux16_d, auxf_d, t1, shard1, h_l,
                        iota_sb, ident_sb, TS, 1)

            # ---- phase A2: h = elu(h_l); h @ W2; out prefill ----
            for j in range(NBLK // 2):
                k0 = 2 * j
                hk = sba.tile([128, 2, OUT], F32, tag="hk")
                nc.sync.dma_start(
                    hk[:],
                    h_l[k0 * 128 : (k0 + 2) * 128, :].rearrange(
                        "(j p) w -> p j w", j=2))
                hm = sba.tile([128, 2, OUT], F32, tag="hm")
                nc.vector.tensor_scalar_max(
                    hm[:].rearrange("p j w -> p (j w)"),
                    hk[:].rearrange("p j w -> p (j w)"), 0.0)
                m0 = sba.tile([128, 2, OUT], F32, tag="m0")
                nc.vector.tensor_scalar_min(
                    m0[:].rearrange("p j w -> p (j w)"),
                    hk[:].rearrange("p j w -> p (j w)"), 0.0)
                e1 = sba.tile([128, 2, OUT], F32, tag="e1")
                nc.scalar.activation(
                    e1[:].rearrange("p j w -> p (j w)"),
                    m0[:].rearrange("p j w -> p (j w)"), AF.Exp)
                s1 = sba.tile([128, 2, OUT], F32, tag="s1")
                nc.vector.tensor_tensor(
                    s1[:].rearrange("p j w -> p (j w)"),
                    hm[:].rearrange("p j w -> p (j w)"),
                    e1[:].rearrange("p j w -> p (j w)"), OP.add)
                he = sba.tile([128, 2, OUT], BF16, tag="he")
                nc.vector.tensor_scalar_add(
                    he[:].rearrange("p j w -> p (j w)"),
                    s1[:].rearrange("p j w -> p (j w)"), -1.0)
                pa = pp.tile([128, 4, OUT], F32, tag="pa")
                for jj in range(2):
                    pt = pp.tile([OUT, 128], BF16, tag="pt", bufs=1)
                    nc.tensor.transpose(pt[:], he[:, jj, :], ident_sb[:])
                    hT = sba.tile([OUT, 128], BF16, tag="hT")
                    nc.scalar.activation(hT[:], pt[:], AF.Copy)
                    nc.tensor.matmul(pa[:, jj], lhsT=hT[:], rhs=w2_sb[:],
                                     start=True, stop=True)
                sh2 = sba.tile([128, 2, ROW], BF16, tag="sh2")
                nc.scalar.activation(sh2[:, :, 0:OUT], pa[:, 0:2], AF.Copy)
                nc.vector.memset(sh2[:, :, OUT : OUT + 1], 1.0)
                t2l = sba.tile([128, 2, OUT], F32, tag="t2l")
                nc.vector.tensor_tensor(
                    t2l[:], pa[:, 0:2],
                    atl2_sb[:, None, :].broadcast_to([128, 2, OUT]), OP.mult)
                alr2 = sba.tile([128, 2, 2], F32, tag="alr2")
                nc.vector.tensor_reduce(alr2[:, :, 0:1], t2l[:], AX.X, OP.add)
                t2r = sba.tile([128, 2, OUT], F32, tag="t2r")
                nc.vector.tensor_tensor(
                    t2r[:], pa[:, 0:2],
                    atr2_sb[:, None, :].broadcast_to([128, 2, OUT]), OP.mult)
                nc.vector.tensor_reduce(alr2[:, :, 1:2], t2r[:], AX.X, OP.add)
                nc.scalar.activation(sh2[:, :, OUT + 1 : OUT + 3], alr2[:], AF.Copy)
                o2 = sba.tile([128, 2, OUT], F32, tag="o2")
                nc.vector.tensor_tensor(
                    o2[:], pa[:, 0:2],
                    b2_sb[:, None, :].broadcast_to([128, 2, OUT]), OP.add)
                store2(shard2, sh2, k0, ROW)
                store2(out_d, o2, k0, OUT)
                for ci, (blo, bhi) in enumerate(CHB[:-1]):
                    if k0 + 2 == bhi:
                        ag_chunk(shard2, t2, blo, bhi, False, f"2_{ci}")
            nc.sync.dma_start(shard2[NLOC:NPAD, :], zrow_sb[:, 0:ROW])
            ag_chunk(shard2, t2, CHB[-1][0], CHB[-1][1], True, '2_3')

            _edge_layer(nc, sbb, pp, aux16_d, auxf_d, t2, shard2, out_d,
                        iota_sb, ident_sb, TS, 2)

    nc.finalize()
    return nc


def _edge_layer(nc, sbb, pp, aux16_d, auxf_d, table, loc_tab, out_tab,
                iota_sb, ident_sb, TS, layer):
    H = HEADS if layer == 1 else 1
    MW = OUT + H
    CL = OUT + H            # al col offset in table row
    CR = OUT + 2 * H        # ar col offset
    NCH = OUT // H          # x channel chunks per head
    tg = f"l{layer}"
    for s in range(TS):
        aux16t = sbb.tile([128, CA16], I16, tag=f"{tg}a16", bufs=4)
        nc.sync.dma_start(aux16t[:], aux16_d[s, :, :])
        auxft = sbb.tile([128, B * GROUPS], F32, tag=f"{tg}af", bufs=4)
        nc.sync.dma_start(auxft[:], auxf_d[s, :, :])

        Xall = sbb.tile([128, GROUPS, B * R, ROW], BF16, tag=f"{tg}X", bufs=4)
        NH = max(1, (B * R * 128) // 1024)   # ucode limit: 1024 idxs per call
        HW_ = B * R * 128 // NH              # idxs per call
        for g in range(GROUPS):
            for h in range(NH):
                nc.gpsimd.dma_gather(
                    out_ap=Xall[:, g, h * (B * R // NH) : (h + 1) * (B * R // NH), :],
                    in_ap=table[g * QROWS : (g + 1) * QROWS, :],
                    idxs_ap=aux16t[:, g * B * 32 + h * (HW_ // 16) :
                                   g * B * 32 + (h + 1) * (HW_ // 16)],
                    num_idxs=HW_,
                    num_idxs_reg=HW_,
                    elem_size=ROW,
                    queue_num=g,
                )
        ART = sbb.tile([128, B, ROW], BF16, tag=f"{tg}AR", bufs=4)
        nc.gpsimd.dma_gather(
            out_ap=ART[:],
            in_ap=loc_tab[:],
            idxs_ap=aux16t[:, CA_X:CA_AR],
            num_idxs=B * 128,
            num_idxs_reg=B * 128,
            elem_size=ROW,
            queue_num=(s % 4),
        )

        Dbuf = sbb.tile([128, B * H], F32, tag=f"{tg}D")
        Nbuf = sbb.tile([128, B, OUT], F32, tag=f"{tg}N")
        for i in range(B):
            # one-hot slot->dst (all quarters in one op)
            O = sbb.tile([128, GROUPS, 128], BF16, tag=f"{tg}O")
            nc.vector.tensor_tensor(
                O[:],
                iota_sb[:, None, :].broadcast_to([128, GROUPS, 128]),
                auxft[:, i * GROUPS : (i + 1) * GROUPS, None]
                .broadcast_to([128, GROUPS, 128]),
                OP.is_equal,
            )
            arp = pp.tile([128, GROUPS * H], F32, tag="arp", bufs=1)
            for g in range(GROUPS):
                otp = pp.tile([128, 128], BF16, tag="otp")
                nc.tensor.transpose(otp[:], O[:, g, :], ident_sb[:])
                ot = sbb.tile([128, 128], BF16, tag=f"{tg}OT")
                nc.scalar.activation(ot[:], otp[:], AF.Copy)
                nc.tensor.matmul(
                    arp[:, g * H : (g + 1) * H], lhsT=ot[:],
                    rhs=ART[:, i, CR : CR + H], start=True, stop=True,
                )
            # logits: al[src] + ar[dst-of-slot], leaky relu + exp on ACT
            L = sbb.tile([128, GROUPS, R, H], BF16, tag=f"{tg}L")
            nc.vector.tensor_tensor(
                L[:], Xall[:, :, i * R : (i + 1) * R, CL : CL + H],
                arp[:].rearrange("p (g h) -> p g h", h=H)[:, :, None, :]
                .broadcast_to([128, GROUPS, R, H]),
                OP.add,
            )
            LM = sbb.tile([128, GROUPS * R * H], BF16, tag=f"{tg}LM")
            nc.vector.tensor_scalar_mul(
                LM[:], L[:].rearrange("p g r h -> p (g r h)"), SLOPE)
            LR = sbb.tile([128, GROUPS * R * H], BF16, tag=f"{tg}LR")
            nc.vector.tensor_tensor(
                LR[:], L[:].rearrange("p g r h -> p (g r h)"), LM[:], OP.max)
            W = sbb.tile([128, GROUPS, R, H], BF16, tag=f"{tg}W")
            nc.scalar.activation(
                W[:].rearrange("p g r h -> p (g r h)"), LR[:], AF.Exp)
            # messages: [x | ones] * w
            P = pp.tile([128, MW], F32, tag="P")
            for g in range(GROUPS):
                MSG = sbb.tile([128, R, MW], BF16, tag=f"{tg}MSG{g}")
                nc.vector.tensor_tensor(
                    MSG[:].rearrange("p r (c h) -> p r c h", h=H),
                    Xall[:, g, i * R : (i + 1) * R, 0:MW].rearrange(
                        "p r (c h) -> p r c h", h=H),
                    W[:, g, :, None, :].broadcast_to([128, R, NCH + 1, H]),
                    OP.mult,
                )
                for r in range(R):
                    b = R * g + r
                    nc.tensor.matmul(
                        P[:], lhsT=O[:, g, :], rhs=MSG[:, r, :],
                        start=(b == 0), stop=(b == BLOCKS - 1),
                    )
            nc.scalar.activation(
                Dbuf[:, i * H : (i + 1) * H], P[:, OUT:MW], AF.Copy, bias=1e-16)
            nc.scalar.activation(Nbuf[:, i, :], P[:, 0:OUT], AF.Copy)
        Rc = sbb.tile([128, B * H], F32, tag=f"{tg}Rc")
        nc.vector.reciprocal(Rc[:], Dbuf[:])
        AG = sbb.tile([128, B, OUT], F32, tag=f"{tg}AG")
        for i in range(B):
            if H > 1:
                nc.vector.tensor_tensor(
                    AG[:, i].rearrange("p (c h) -> p c h", h=H),
                    Nbuf[:, i].rearrange("p (c h) -> p c h", h=H),
                    Rc[:, None, i * H : (i + 1) * H].broadcast_to([128, NCH, H]),
                    OP.mult,
                )
            else:
                nc.vector.tensor_tensor(
                    AG[:, i], Nbuf[:, i],
                    Rc[:, i : i + 1].broadcast_to([128, OUT]),
                    OP.mult,
                )
        nc.gpsimd.dma_scatter_add(
            out_ap=out_tab[:],
            in_ap=AG[:],
            idxs_ap=aux16t[:, CA_AR:CA_OUT],
            num_idxs=B * 128,
            num_idxs_reg=B * 128,
            elem_size=OUT,
            queue_num=((s + 2) % 4),
        )
# --------------------------------------------------------------------------
# entry point
# --------------------------------------------------------------------------

def _run(inputs):
    x = np.asarray(inputs["x"], np.float32)
    edge_index = np.asarray(inputs["edge_index"], np.int32)
    W1 = np.asarray(inputs["W1"], np.float32)
    att_l1 = np.asarray(inputs["att_l1"], np.float32)
    att_r1 = np.asarray(inputs["att_r1"], np.float32)
    res_W1 = np.asarray(inputs["res_W1"], np.float32)
    b1 = np.asarray(inputs["b1"], np.float32).reshape(-1)
    W2 = np.asarray(inputs["W2"], np.float32)
    att_l2 = np.asarray(inputs["att_l2"], np.float32)
    att_r2 = np.asarray(inputs["att_r2"], np.float32)
    b2 = np.asarray(inputs["b2"], np.float32).reshape(-1)

    a16, af, TS = _prepare(edge_index)

    perm = np.arange(OUT).reshape(HEADS, HID).T.reshape(-1)
    W1p = np.ascontiguousarray(W1[:, perm])
    rw1p = np.ascontiguousarray(res_W1[:, perm])
    atl1_cm = att_l1.reshape(HEADS, HID).T.reshape(1, OUT)
    atr1_cm = att_r1.reshape(HEADS, HID).T.reshape(1, OUT)
    b1p = b1[perm].reshape(1, OUT)
    W2p = np.ascontiguousarray(W2[perm, :])

    iota = np.broadcast_to(np.arange(128, dtype=np.float32), (128, 128)).copy()
    ident = np.eye(128, dtype=np.float32)

    if TS not in _CACHE:
        _CACHE[TS] = _build(TS)
    nc = _CACHE[TS]

    def bf(a):
        import ml_dtypes
        return a.astype(ml_dtypes.bfloat16)

    in_maps = []
    for c in range(NC):
        xTc = np.zeros((NBLK * 128, 128), np.float32)
        xl = x[c * NLOC : (c + 1) * NLOC]
        for bb in range(NBLK):
            blk = xl[bb * 128 : (bb + 1) * 128]
            xTc[bb * 128 : bb * 128 + 128, : blk.shape[0]] = blk.T
        in_maps.append(
            {
                "xT": bf(xTc),
                "aux16": a16[c],
                "auxf": af[c],
                "w1": bf(W1p),
                "rw1": bf(rw1p),
                "atl1": np.tile(atl1_cm, (128, 1)),
                "atr1": np.tile(atr1_cm, (128, 1)),
                "b1": np.tile(b1p, (128, 1)),
                "w2": bf(W2p),
                "atl2": np.tile(att_l2.reshape(1, OUT), (128, 1)),
                "atr2": np.tile(att_r2.reshape(1, OUT), (128, 1)),
                "b2": np.tile(b2.reshape(1, OUT), (128, 1)),
                "iota": iota,
                "ident": bf(ident),
            }
        )
    return nc, in_maps


def kernel(**inputs):
    nc, in_maps = _run(inputs)
    res = bass_utils.run_bass_kernel_spmd(
        nc, in_maps, core_ids=list(range(NC)), trace=False
    )
    return np.concatenate(
        [res.results[c]["out"][:NLOC] for c in range(NC)], axis=0
    )


# revision 17
# speedup vs baseline: 1.0140x; 1.0140x over previous
"""AGDN (2-layer GAT-style message passing) distributed Bass kernel for 8 TRN2 cores.

v2: dst-sharded edge processing, bf16 gather tables (256B rows), batched
dma_gather calls (B tiles per ucode launch), attention-r broadcast via PE
transpose + one-hot matmul (no per-slot gather), residual+bias prefilled in
DRAM with a dma_scatter_add epilogue (no per-tile residual gather, no
indirect scatter).

Table row (128 bf16 = 256B): [x_lin(64, head-interleaved) | ones(H) | al(H) |
ar(H) | pad].  Edges are packed host-side into R=4 slots sharing (dst,
src-quarter); tiles cover <=128 dst nodes x 4 quarters x 128 slots.
"""

import numpy as np

import concourse.bass as bass
import concourse.bacc as bacc
import concourse.mybir as mybir
import concourse.tile as tile
from concourse import bass_utils

F32 = mybir.dt.float32
BF16 = mybir.dt.bfloat16
I16 = mybir.dt.int16
AF = mybir.ActivationFunctionType
OP = mybir.AluOpType
AX = mybir.AxisListType

# problem constants
IN, HID, HEADS, OUT = 128, 16, 4, 64
SLOPE = 0.2
NC = 8
NQ = 4                  # src quarters (2 cores each)
R = 4                   # edges per slot (same dst, same src-quarter)
GROUPS = 4
BLOCKS = GROUPS * R
ROW = 128               # table row bf16 elems (256B)
B = 4                   # tiles per super-tile (gather batching)

_CACHE = {}


def _set_dims(n, e):
    global N, E, NLOC, NPAD, HBLK, NBLK, QROWS
    N, E = n, e
    NLOC = N // NC
    NPAD = NLOC + 4
    HBLK = ((NPAD + 127) // 128) * 128
    NBLK = HBLK // 128
    QROWS = 2 * NPAD
    assert QROWS < 32768


_set_dims(100000, 1600000)

# aux16 column layout (per super-tile)
CA_X = 4 * B * 32                 # X gather idxs, quarter-major
CA_AR = CA_X + B * 8              # ar-tile gather idxs
CA_OUT = CA_AR + B * 8            # scatter-add out idxs
CA16 = CA_OUT


def _wrap16(idx):
    """[n] -> int16 [128, n//16] replicated-8 layout for dma_gather."""
    n = idx.shape[0]
    blk = idx.reshape(n // 16, 16).T.astype(np.int16)
    return np.tile(blk, (8, 1))


# --------------------------------------------------------------------------
# host-side graph packing
# --------------------------------------------------------------------------

def _pack_core(src_g, dst_l):
    quarter = src_g // (2 * NLOC)
    key = dst_l * NQ + quarter
    korder = np.argsort(key, kind="stable")
    src_g = src_g[korder]
    key = key[korder]
    cnt = np.bincount(key, minlength=NLOC * NQ)
    estart = np.concatenate([[0], np.cumsum(cnt)])[:-1]
    nslot_nq = ((cnt + R - 1) // R).reshape(NLOC, NQ)

    cum_q = np.cumsum(nslot_nq, axis=0)
    tiles = []
    n0 = 0
    base = np.zeros(NQ, np.int64)
    while n0 < NLOC:
        n_hi = min(n0 + 128, NLOC)
        ok = ((cum_q[n0:n_hi] - base[None, :]) <= 128).all(axis=1)
        k = int(np.argmin(ok)) if not ok.all() else n_hi - n0
        n1 = n0 + max(k, 1)
        if (nslot_nq[n0] > 128).any():
            raise ValueError("node with too many edges in one quarter")
        tiles.append((n0, n1))
        base = cum_q[n1 - 1].copy()
        n0 = n1
    T = len(tiles)

    zq = NLOC   # quarter-local / local zero row
    srcidx = np.full((T, GROUPS, R, 128), zq, np.int64)
    ld = np.full((T, 128, GROUPS), 255.0, np.float32)
    outl = np.zeros((T, 128), np.int64)
    for t, (nlo, nhi) in enumerate(tiles):
        for g in range(NQ):
            p = 0
            for d in range(nlo, nhi):
                ns = nslot_nq[d, g]
                if ns == 0:
                    continue
                e0 = estart[d * NQ + g]
                ne = cnt[d * NQ + g]
                for s in range(ns):
                    tk = min(R, ne - R * s)
                    rows = src_g[e0 + R * s : e0 + R * s + tk]
                    srcidx[t, g, :tk, p] = (rows // NLOC % 2) * NPAD + rows % NLOC
                    ld[t, p, g] = d - nlo
                    p += 1
            assert p <= 128
        out_l = np.arange(128) + nlo
        pad = out_l >= nhi
        out_l[pad] = NLOC + (np.arange(128)[pad] % 4)
        outl[t] = out_l
    return srcidx, ld, outl, T


def _prepare(edge_index):
    src = edge_index[0].astype(np.int64)
    dst = edge_index[1].astype(np.int64)
    core = dst // NLOC
    packs = []
    for c in range(NC):
        m = core == c
        packs.append(_pack_core(src[m], dst[m] - c * NLOC))
    Tmax = max(p[3] for p in packs)
    TS = (Tmax + B - 1) // B
    Tp = TS * B
    a16 = np.zeros((NC, TS, 128, CA16), np.int16)
    af = np.zeros((NC, TS, 128, B * GROUPS), np.float32)
    for c in range(NC):
        srcidx, ld, outl, T = packs[c]
        # pad to Tp tiles
        si = np.full((Tp, GROUPS, R, 128), NLOC, np.int64)
        lf = np.full((Tp, 128, GROUPS), 255.0, np.float32)
        ol = np.full((Tp, 128), NLOC, np.int64)
        si[:T] = srcidx
        lf[:T] = ld
        ol[:T] = outl
        for s in range(TS):
            for g in range(GROUPS):
                # idx order: flat j = (i*R + r)*128 + p  -> out[p, i*R+r, :]
                flat = si[s * B : (s + 1) * B, g].reshape(-1)
                a16[c, s, :, g * B * 32 : (g + 1) * B * 32] = _wrap16(flat)
            a16[c, s, :, CA_X:CA_AR] = _wrap16(ol[s * B : (s + 1) * B].reshape(-1))
            a16[c, s, :, CA_AR:CA_OUT] = _wrap16(ol[s * B : (s + 1) * B].reshape(-1))
            af[c, s] = lf[s * B : (s + 1) * B].transpose(1, 0, 2).reshape(128, B * GROUPS)
    return a16, af, TS


# --------------------------------------------------------------------------
# device kernel
# --------------------------------------------------------------------------

def _build(TS):
    nc = bacc.Bacc("TRN2", target_bir_lowering=False, num_devices=NC,
                   num_swdge_queues=4)

    xT = nc.declare_dram_parameter("xT", [NBLK * 128, 128], BF16, isOutput=False)
    aux16_d = nc.declare_dram_parameter("aux16", [TS, 128, CA16], I16, isOutput=False)
    auxf_d = nc.declare_dram_parameter("auxf", [TS, 128, B * GROUPS], F32, isOutput=False)
    w1 = nc.declare_dram_parameter("w1", [IN, OUT], BF16, isOutput=False)
    rw1 = nc.declare_dram_parameter("rw1", [IN, OUT], BF16, isOutput=False)
    atl1 = nc.declare_dram_parameter("atl1", [128, OUT], F32, isOutput=False)
    atr1 = nc.declare_dram_parameter("atr1", [128, OUT], F32, isOutput=False)
    b1 = nc.declare_dram_parameter("b1", [128, OUT], F32, isOutput=False)
    w2 = nc.declare_dram_parameter("w2", [OUT, OUT], BF16, isOutput=False)
    atl2 = nc.declare_dram_parameter("atl2", [128, OUT], F32, isOutput=False)
    atr2 = nc.declare_dram_parameter("atr2", [128, OUT], F32, isOutput=False)
    b2 = nc.declare_dram_parameter("b2", [128, OUT], F32, isOutput=False)
    iota_d = nc.declare_dram_parameter("iota", [128, 128], F32, isOutput=False)
    ident_d = nc.declare_dram_parameter("ident", [128, 128], BF16, isOutput=False)
    out_d = nc.declare_dram_parameter("out", [NPAD, OUT], F32, isOutput=True)

    CHB = [(0, 24), (24, 48), (48, 72), (72, NBLK)]   # AllGather chunks (blocks)

    with tile.TileContext(nc) as tc:
        with (
            tc.tile_pool(name="dram", bufs=1, space="DRAM") as dram,
            tc.tile_pool(name="const", bufs=1) as cpool,
            tc.tile_pool(name="sba", bufs=3) as sba,
            tc.tile_pool(name="psum", bufs=2, space="PSUM") as pp,
            tc.tile_pool(name="sbb", bufs=3) as sbb,
        ):
            shard1 = dram.tile([NPAD, ROW], BF16)
            t1 = dram.tile([NC * NPAD, ROW], BF16)
            h_l = dram.tile([HBLK, OUT], F32)
            shard2 = dram.tile([NPAD, ROW], BF16)
            t2 = dram.tile([NC * NPAD, ROW], BF16)

            def const_from(handle, shape, tag, dt=F32):
                t_ = cpool.tile(shape, dt, tag=tag, name=tag)
                nc.sync.dma_start(t_[:], handle[:])
                return t_

            w1_sb = const_from(w1, [IN, OUT], "c_w1", BF16)
            rw1_sb = const_from(rw1, [IN, OUT], "c_rw1", BF16)
            w2_sb = const_from(w2, [OUT, OUT], "c_w2", BF16)
            atl1_sb = const_from(atl1, [128, OUT], "c_atl1")
            atr1_sb = const_from(atr1, [128, OUT], "c_atr1")
            b1_sb = const_from(b1, [128, OUT], "c_b1")
            atl2_sb = const_from(atl2, [128, OUT], "c_atl2")
            atr2_sb = const_from(atr2, [128, OUT], "c_atr2")
            b2_sb = const_from(b2, [128, OUT], "c_b2")
            iota_sb = const_from(iota_d, [128, 128], "c_iota")
            ident_sb = const_from(ident_d, [128, 128], "c_ident", BF16)
            zrow_sb = cpool.tile([4, ROW], BF16)
            nc.vector.memset(zrow_sb[:], 0.0)
            ztail_sb = cpool.tile([HBLK - NLOC, OUT], F32)
            nc.vector.memset(ztail_sb[:], 0.0)

            def ag_chunk(shard, tdst, blo, bhi, last, tag):
                a = blo * 128
                b_ = NPAD if last else bhi * 128
                rows = b_ - a
                tmp = dram.tile([NC * rows, ROW], BF16, tag=f"agt{tag}",
                                name=f"agt{tag}")
                nc.gpsimd.collective_compute(
                    "AllGather", OP.bypass,
                    replica_groups=[list(range(NC))],
                    ins=[shard[a:b_, :].opt()], outs=[tmp.opt()],
                )
                for c in range(NC):
                    nc.sync.dma_start(
                        tdst[c * NPAD + a : c * NPAD + b_, :],
                        tmp[c * rows : (c + 1) * rows, :])

            def store2(dst, buf, k0, width):
                # buf [128, 2, width] -> dst rows [k0*128 : k0*128+256) (clip NLOC)
                lo = k0 * 128
                if lo + 256 <= NLOC:
                    nc.sync.dma_start(
                        dst[lo : lo + 256, :].rearrange("(j p) w -> p j w", j=2),
                        buf[:],
                    )
                else:
                    r0 = min(128, NLOC - lo)
                    nc.sync.dma_start(dst[lo : lo + r0, :], buf[:r0, 0, :])
                    r1 = min(128, NLOC - lo - 128)
                    if r1 > 0:
                        nc.sync.dma_start(
                            dst[lo + 128 : lo + 128 + r1, :], buf[:r1, 1, :])

            # ---- phase A: x @ W1, attention dots, residual prefill ----
            for k in range(NBLK):
                rows = min(128, NLOC - 128 * k)
                if rows <= 0:
                    break
                xk = sba.tile([128, 128], BF16, tag="xk")
                nc.sync.dma_start(xk[:], xT[128 * k : 128 * (k + 1), :])
                p0 = pp.tile([128, OUT], F32, tag="pa")
                nc.tensor.matmul(p0[:], lhsT=xk[:], rhs=w1_sb[:], start=True, stop=True)
                p1 = pp.tile([128, OUT], F32, tag="pa")
                nc.tensor.matmul(p1[:], lhsT=xk[:], rhs=rw1_sb[:], start=True, stop=True)
                sh = sba.tile([128, ROW], BF16, tag="sh")
                nc.scalar.activation(sh[:, 0:OUT], p0[:], AF.Copy)
                nc.vector.memset(sh[:, OUT : OUT + HEADS], 1.0)
                tal = sba.tile([128, OUT], F32, tag="tal")
                nc.vector.tensor_tensor(tal[:], p0[:], atl1_sb[:], OP.mult)
                alr = sba.tile([128, 2 * HEADS], F32, tag="alr")
                nc.vector.tensor_reduce(
                    alr[:, 0:HEADS],
                    tal[:].rearrange("p (c h) -> p h c", h=HEADS),
                    AX.X, OP.add,
                )
                tar = sba.tile([128, OUT], F32, tag="tar")
                nc.vector.tensor_tensor(tar[:], p0[:], atr1_sb[:], OP.mult)
                nc.vector.tensor_reduce(
                    alr[:, HEADS : 2 * HEADS],
                    tar[:].rearrange("p (c h) -> p h c", h=HEADS),
                    AX.X, OP.add,
                )
                nc.scalar.activation(sh[:, OUT + HEADS : OUT + 3 * HEADS], alr[:], AF.Copy)
                xr = sba.tile([128, OUT], F32, tag="xr")
                nc.vector.tensor_tensor(xr[:], p1[:], b1_sb[:], OP.add)
                lo = 128 * k
                nc.sync.dma_start(shard1[lo : lo + rows, :], sh[:rows, :])
                nc.sync.dma_start(h_l[lo : lo + rows, :], xr[:rows, :])
                for ci, (blo, bhi) in enumerate(CHB[:-1]):
                    if k + 1 == bhi:
                        ag_chunk(shard1, t1, blo, bhi, False, f"1_{ci}")
            nc.sync.dma_start(shard1[NLOC:NPAD, :], zrow_sb[:, 0:ROW])
            nc.sync.dma_start(h_l[NLOC:HBLK, :], ztail_sb[:])
            ag_chunk(shard1, t1, CHB[-1][0], CHB[-1][1], True, '1_3')

            _edge_layer(nc, sbb, pp, aux16_d, auxf_d, t1, shard1, h_l,
                        iota_sb, ident_sb, TS, 1)

            # ---- phase A2: h = elu(h_l); h @ W2; out prefill ----
            for k in range(NBLK):
                rows = min(128, NLOC - 128 * k)
                hk = sba.tile([128, OUT], F32, tag="hk")
                nc.sync.dma_start(hk[:], h_l[128 * k : 128 * (k + 1), :])
                hm = sba.tile([128, OUT], F32, tag="hm")
                nc.vector.tensor_scalar_max(hm[:], hk[:], 0.0)
                m0 = sba.tile([128, OUT], F32, tag="m0")
                nc.vector.tensor_scalar_min(m0[:], hk[:], 0.0)
                e1 = sba.tile([128, OUT], F32, tag="e1")
                nc.scalar.activation(e1[:], m0[:], AF.Exp)
                s1 = sba.tile([128, OUT], F32, tag="s1")
                nc.vector.tensor_tensor(s1[:], hm[:], e1[:], OP.add)
                he = sba.tile([128, OUT], BF16, tag="he")
                nc.vector.tensor_scalar_add(he[:], s1[:], -1.0)
                pt = pp.tile([OUT, 128], BF16, tag="pt", bufs=1)
                nc.tensor.transpose(pt[:], he[:], ident_sb[:])
                hT = sba.tile([OUT, 128], BF16, tag="hT")
                nc.scalar.activation(hT[:], pt[:], AF.Copy)
                p2 = pp.tile([128, OUT], F32, tag="pa")
                nc.tensor.matmul(p2[:], lhsT=hT[:], rhs=w2_sb[:], start=True, stop=True)
                if rows <= 0:
                    continue
                sh2 = sba.tile([128, ROW], BF16, tag="sh2")
                nc.scalar.activation(sh2[:, 0:OUT], p2[:], AF.Copy)
                nc.vector.memset(sh2[:, OUT : OUT + 1], 1.0)
                t2l = sba.tile([128, OUT], F32, tag="t2l")
                nc.vector.tensor_tensor(t2l[:], p2[:], atl2_sb[:], OP.mult)
                alr2 = sba.tile([128, 2], F32, tag="alr2")
                nc.vector.tensor_reduce(alr2[:, 0:1], t2l[:], AX.X, OP.add)
                t2r = sba.tile([128, OUT], F32, tag="t2r")
                nc.vector.tensor_tensor(t2r[:], p2[:], atr2_sb[:], OP.mult)
                nc.vector.tensor_reduce(alr2[:, 1:2], t2r[:], AX.X, OP.add)
                nc.scalar.activation(sh2[:, OUT + 1 : OUT + 3], alr2[:], AF.Copy)
                o2 = sba.tile([128, OUT], F32, tag="o2")
                nc.vector.tensor_tensor(o2[:], p2[:], b2_sb[:], OP.add)
                lo = 128 * k
                nc.sync.dma_start(shard2[lo : lo + rows, :], sh2[:rows, :])
                nc.sync.dma_start(out_d[lo : lo + rows, :], o2[:rows, :])
                for ci, (blo, bhi) in enumerate(CHB[:-1]):
                    if k + 1 == bhi:
                        ag_chunk(shard2, t2, blo, bhi, False, f"2_{ci}")
            nc.sync.dma_start(shard2[NLOC:NPAD, :], zrow_sb[:, 0:ROW])
            ag_chunk(shard2, t2, CHB[-1][0], CHB[-1][1], True, '2_3')

            _edge_layer(nc, sbb, pp, aux16_d, auxf_d, t2, shard2, out_d,
                        iota_sb, ident_sb, TS, 2)

    nc.finalize()
    return nc


def _edge_layer(nc, sbb, pp, aux16_d, auxf_d, table, loc_tab, out_tab,
                iota_sb, ident_sb, TS, layer):
    H = HEADS if layer == 1 else 1
    MW = OUT + H
    CL = OUT + H            # al col offset in table row
    CR = OUT + 2 * H        # ar col offset
    NCH = OUT // H          # x channel chunks per head
    tg = f"l{layer}"
    pend = []
    for s in range(TS):
        aux16t = sbb.tile([128, CA16], I16, tag=f"{tg}a16", bufs=4)
        nc.sync.dma_start(aux16t[:], aux16_d[s, :, :])
        auxft = sbb.tile([128, B * GROUPS], F32, tag=f"{tg}af", bufs=4)
        nc.sync.dma_start(auxft[:], auxf_d[s, :, :])

        Xall = sbb.tile([128, GROUPS, B * R, ROW], BF16, tag=f"{tg}X", bufs=4)
        NH = max(1, (B * R * 128) // 1024)   # ucode limit: 1024 idxs per call
        HW_ = B * R * 128 // NH              # idxs per call
        for g in range(GROUPS):
            for h in range(NH):
                nc.gpsimd.dma_gather(
                    out_ap=Xall[:, g, h * (B * R // NH) : (h + 1) * (B * R // NH), :],
                    in_ap=table[g * QROWS : (g + 1) * QROWS, :],
                    idxs_ap=aux16t[:, g * B * 32 + h * (HW_ // 16) :
                                   g * B * 32 + (h + 1) * (HW_ // 16)],
                    num_idxs=HW_,
                    num_idxs_reg=HW_,
                    elem_size=ROW,
                    queue_num=g,
                )
        ART = sbb.tile([128, B, ROW], BF16, tag=f"{tg}AR", bufs=4)
        nc.gpsimd.dma_gather(
            out_ap=ART[:],
            in_ap=loc_tab[:],
            idxs_ap=aux16t[:, CA_X:CA_AR],
            num_idxs=B * 128,
            num_idxs_reg=B * 128,
            elem_size=ROW,
            queue_num=(s % 4),
        )

        for fn in pend:
            fn()
        pend = []

        Dbuf = sbb.tile([128, B * H], F32, tag=f"{tg}D")
        Nbuf = sbb.tile([128, B, OUT], F32, tag=f"{tg}N")
        for i in range(B):
            # one-hot slot->dst (all quarters in one op)
            O = sbb.tile([128, GROUPS, 128], BF16, tag=f"{tg}O")
            nc.vector.tensor_tensor(
                O[:],
                iota_sb[:, None, :].broadcast_to([128, GROUPS, 128]),
                auxft[:, i * GROUPS : (i + 1) * GROUPS, None]
                .broadcast_to([128, GROUPS, 128]),
                OP.is_equal,
            )
            arp = pp.tile([128, GROUPS * H], F32, tag="arp", bufs=1)
            for g in range(GROUPS):
                otp = pp.tile([128, 128], BF16, tag="otp")
                nc.tensor.transpose(otp[:], O[:, g, :], ident_sb[:])
                ot = sbb.tile([128, 128], BF16, tag=f"{tg}OT")
                nc.scalar.activation(ot[:], otp[:], AF.Copy)
                nc.tensor.matmul(
                    arp[:, g * H : (g + 1) * H], lhsT=ot[:],
                    rhs=ART[:, i, CR : CR + H], start=True, stop=True,
                )
            # logits: al[src] + ar[dst-of-slot], leaky relu + exp on ACT
            L = sbb.tile([128, GROUPS, R, H], BF16, tag=f"{tg}L")
            nc.vector.tensor_tensor(
                L[:], Xall[:, :, i * R : (i + 1) * R, CL : CL + H],
                arp[:].rearrange("p (g h) -> p g h", h=H)[:, :, None, :]
                .broadcast_to([128, GROUPS, R, H]),
                OP.add,
            )
            LM = sbb.tile([128, GROUPS * R * H], BF16, tag=f"{tg}LM")
            nc.vector.tensor_scalar_mul(
                LM[:], L[:].rearrange("p g r h -> p (g r h)"), SLOPE)
            LR = sbb.tile([128, GROUPS * R * H], BF16, tag=f"{tg}LR")
            nc.vector.tensor_tensor(
                LR[:], L[:].rearrange("p g r h -> p (g r h)"), LM[:], OP.max)
            W = sbb.tile([128, GROUPS, R, H], BF16, tag=f"{tg}W")
            nc.scalar.activation(
                W[:].rearrange("p g r h -> p (g r h)"), LR[:], AF.Exp)
            # messages: [x | ones] * w
            P = pp.tile([128, MW], F32, tag="P")
            for g in range(GROUPS):
                MSG = sbb.tile([128, R, MW], BF16, tag=f"{tg}MSG{g}")
                nc.vector.tensor_tensor(
                    MSG[:].rearrange("p r (c h) -> p r c h", h=H),
                    Xall[:, g, i * R : (i + 1) * R, 0:MW].rearrange(
                        "p r (c h) -> p r c h", h=H),
                    W[:, g, :, None, :].broadcast_to([128, R, NCH + 1, H]),
                    OP.mult,
                )
                for r in range(R):
                    b = R * g + r
                    nc.tensor.matmul(
                        P[:], lhsT=O[:, g, :], rhs=MSG[:, r, :],
                        start=(b == 0), stop=(b == BLOCKS - 1),
                    )
            nc.scalar.activation(
                Dbuf[:, i * H : (i + 1) * H], P[:, OUT:MW], AF.Copy, bias=1e-16)
            nc.scalar.activation(Nbuf[:, i, :], P[:, 0:OUT], AF.Copy)
        Rc = sbb.tile([128, B * H], F32, tag=f"{tg}Rc")
        nc.vector.reciprocal(Rc[:], Dbuf[:])
        AG = sbb.tile([128, B, OUT], F32, tag=f"{tg}AG")
        for i in range(B):
            if H > 1:
                nc.vector.tensor_tensor(
                    AG[:, i].rearrange("p (c h) -> p c h", h=H),
                    Nbuf[:, i].rearrange("p (c h) -> p c h", h=H),
                    Rc[:, None, i * H : (i + 1) * H].broadcast_to([128, NCH, H]),
                    OP.mult,
                )
            else:
                nc.vector.tensor_tensor(
                    AG[:, i], Nbuf[:, i],
                    Rc[:, i : i + 1].broadcast_to([128, OUT]),
                    OP.mult,
                )
        def _scat(AG=AG, aux16t=aux16t, s=s):
            nc.gpsimd.dma_scatter_add(
                out_ap=out_tab[:],
                in_ap=AG[:],
                idxs_ap=aux16t[:, CA_AR:CA_OUT],
                num_idxs=B * 128,
                num_idxs_reg=B * 128,
                elem_size=OUT,
                queue_num=((s + 2) % 4),
            )
        pend.append(_scat)
    for fn in pend:
        fn()
# --------------------------------------------------------------------------
# entry point
# --------------------------------------------------------------------------

def _run(inputs):
    x = np.asarray(inputs["x"], np.float32)
    edge_index = np.asarray(inputs["edge_index"], np.int32)
    W1 = np.asarray(inputs["W1"], np.float32)
    att_l1 = np.asarray(inputs["att_l1"], np.float32)
    att_r1 = np.asarray(inputs["att_r1"], np.float32)
    res_W1 = np.asarray(inputs["res_W1"], np.float32)
    b1 = np.asarray(inputs["b1"], np.float32).reshape(-1)
    W2 = np.asarray(inputs["W2"], np.float32)
    att_l2 = np.asarray(inputs["att_l2"], np.float32)
    att_r2 = np.asarray(inputs["att_r2"], np.float32)
    b2 = np.asarray(inputs["b2"], np.float32).reshape(-1)

    a16, af, TS = _prepare(edge_index)

    perm = np.arange(OUT).reshape(HEADS, HID).T.reshape(-1)
    W1p = np.ascontiguousarray(W1[:, perm])
    rw1p = np.ascontiguousarray(res_W1[:, perm])
    atl1_cm = att_l1.reshape(HEADS, HID).T.reshape(1, OUT)
    atr1_cm = att_r1.reshape(HEADS, HID).T.reshape(1, OUT)
    b1p = b1[perm].reshape(1, OUT)
    W2p = np.ascontiguousarray(W2[perm, :])

    iota = np.broadcast_to(np.arange(128, dtype=np.float32), (128, 128)).copy()
    ident = np.eye(128, dtype=np.float32)

    if TS not in _CACHE:
        _CACHE[TS] = _build(TS)
    nc = _CACHE[TS]

    def bf(a):
        import ml_dtypes
        return a.astype(ml_dtypes.bfloat16)

    in_maps = []
    for c in range(NC):
        xTc = np.zeros((NBLK * 128, 128), np.float32)
        xl = x[c * NLOC : (c + 1) * NLOC]
        for bb in range(NBLK):
            blk = xl[bb * 128 : (bb + 1) * 128]
            xTc[bb * 128 : bb * 128 + 128, : blk.shape[0]] = blk.T
        in_maps.append(
            {
                "xT": bf(xTc),
                "aux16": a16[c],
                "auxf": af[c],
                "w1": bf(W1p),
                "rw1": bf(rw1p),
                "atl1": np.tile(atl1_cm, (128, 1)),
                "atr1": np.tile(atr1_cm, (128, 1)),
                "b1": np.tile(b1p, (128, 1)),
                "w2": bf(W2p),
                "atl2": np.tile(att_l2.reshape(1, OUT), (128, 1)),
                "atr2": np.tile(att_r2.reshape(1, OUT), (128, 1)),
                "b2": np.tile(b2.reshape(1, OUT), (128, 1)),
                "iota": iota,
                "ident": bf(ident),
            }
        )
    return nc, in_maps


def kernel(**inputs):
    nc, in_maps = _run(inputs)
    res = bass_utils.run_bass_kernel_spmd(
        nc, in_maps, core_ids=list(range(NC)), trace=False
    )
    return np.concatenate(
        [res.results[c]["out"][:NLOC] for c in range(NC)], axis=0
    )
